# revision 19
# baseline (speedup 1.0000x reference)
"""Dual-score causal attention on 8 Trainium2 NeuronCores.

Math (per batch*head):
    S = (q @ k.T + pe_q @ pe_k.T) * D**-0.5   == concat(q,pe_q) @ concat(k,pe_k).T * scale
    O = softmax(causal_mask(S)) @ v

Sharding: B*H = 32 pairs -> 4 per core (head/data parallel, no collectives).

Design (v6):
  - All layout work happens on the HOST: Q' = [q|pe_q] and K' = [k|pe_k] are
    concatenated, cast f16 and pre-TRANSPOSED to d-major so the device does
    zero transposes and only fully-contiguous HWDGE DMA loads.  V is packed
    [128, NKB, D+1] f16 with a ones column (row-sum trick).
  - S^T tiles [128 k x 512 q]: contraction (d'=128) and the A@V contraction
    both run over the partition axis; the ones column of V' yields softmax
    denominators from the same matmul chain.  Causally-dead k-blocks are
    skipped; diagonal blocks trim dead query columns.
  - ~28 warm-up matmuls on a memset scratch tile run during the DMA-wait
    window: they latch the PE_HAM activity monitor to 8/8 (2.4 GHz) before
    real work arrives, avoiding ~8us of half-clock matmuls.
  - exp() is split across two engines: a deficit balancer assigns each tile
    to exact-exp on ScalarE (bias ln4 matches the 2^2 Schraudolph offset) or
    Schraudolph fast-exp on VectorE (one tensor_scalar x*A+B, int16-convert
    output bit-viewed as f16: bits = round(1024*(log2e*scale*s+17)) =>
    2^t*(1+-3%)).  Only qi=0's diagonal tiles are forced exact: few-term
    softmax rows (q<512) need exact weights, elsewhere the noise averages
    out (4.8e-3 measured vs the 2e-2 gate).
  - The 128-col causal triangles are zeroed by 0/1 f16 multiplies on the
    otherwise-idle GpSimd engine.
  - Output: unnormalized O^T [65, 512] per query block (row 64 = softmax
    denominator) is copied f32->f16 (ScalarE) and DMA'd out contiguously;
    the host divides and transposes.  Epilogues are deferred one stage into
    the next query block so they never head-block the scalar engine queue.
  - The first input DMAs issue from the scalar HWDGE queue (its preamble
    clears ~3us before the sync engine's); bulk loads stay on sync.
"""

import os
import sys

import numpy as np

B, H, L, D = 2, 16, 2048, 64
NCORES = 8
BHPC = (B * H) // NCORES  # bh pairs per core = 4
QB = 512  # query block (S^T free dim)
KB = 128  # key block (S^T partition dim)
NQB = L // QB  # 4
NKB = L // KB  # 16
KB_PER_QB = QB // KB  # 4
SCALE = float(D) ** -0.5
LOG2E = 1.4426950408889634
# Schraudolph f16: bits = round(s * SCHR_A + SCHR_B); exponent offset C=2
# (all weights scaled by 4; cancels in the softmax division on host).
SCHR_A = SCALE * LOG2E * 1024.0
SCHR_B = 1024.0 * (15.0 + 2.0) - 44.2
LN4 = 1.3862943611198906  # exact-exp path matches the 2^2 offset

_CACHE = {}


def _import_concourse():
    try:
        import concourse  # noqa: F401
    except ImportError:
        for p in ("/opt/trn_rl_repo", "/root/.axon_site/_ro/trn_rl_repo"):
            if os.path.isdir(p) and p not in sys.path:
                sys.path.insert(0, p)


def _build_nc():
    """Build the single-core Bass program (same NEFF for all 8 cores)."""
    _import_concourse()
    from contextlib import ExitStack

    import concourse.tile as tile
    from concourse import bacc, mybir

    f32 = mybir.dt.float32
    f16 = mybir.dt.float16
    i16 = mybir.dt.int16
    nc = bacc.Bacc("TRN2", target_bir_lowering=False, debug=False)

    qT_d = nc.dram_tensor("qT", [BHPC, 128, L], f16, kind="ExternalInput").ap()
    kT_d = nc.dram_tensor("kT", [BHPC, 128, L], f16, kind="ExternalInput").ap()
    vp_d = nc.dram_tensor("vp", [BHPC, 128, NKB * (D + 1)], f16, kind="ExternalInput").ap()
    tri_d = nc.dram_tensor("tri", [128, 128], f16, kind="ExternalInput").ap()
    out_d = nc.dram_tensor("out", [BHPC, NQB, D + 1, QB], f16, kind="ExternalOutput").ap()

    Exp = mybir.ActivationFunctionType.Exp
    mult = mybir.AluOpType.mult
    add = mybir.AluOpType.add

    with tile.TileContext(nc) as tc:
        with ExitStack() as ctx:
            ep = ctx.enter_context

            const_pool = ep(tc.tile_pool(name="const", bufs=1))
            qT_pool = ep(tc.tile_pool(name="qT", bufs=2))
            kT_pool = ep(tc.tile_pool(name="kT", bufs=2))
            vp_pool = ep(tc.tile_pool(name="vp", bufs=2))
            ex_pool = ep(tc.tile_pool(name="ex", bufs=7))
            osb_pool = ep(tc.tile_pool(name="osb", bufs=3))
            stp_pool = ep(tc.tile_pool(name="stp", bufs=3, space="PSUM"))
            otp_pool = ep(tc.tile_pool(name="otp", bufs=2, space="PSUM"))

            tri = const_pool.tile([128, 128], f16)
            ln4 = const_pool.tile([128, 1], f32)
            nc.vector.memset(ln4[:], LN4)

            # ---- PE warm-up: latch HAM to 8/8 while DMAs are in flight ----
            scratch = const_pool.tile([128, 128], f16)
            nc.gpsimd.memset(scratch[:], 0.0)
            wup = stp_pool.tile([128, 2 * QB], f32, tag="stp")
            for w in range(64):
                nc.tensor.matmul(
                    wup[:, 0:128],
                    lhsT=scratch[:],
                    rhs=scratch[:],
                    start=True,
                    stop=True,
                    skip_group_check=True,
                )

            # deficit balancer for exp engine assignment (us of est. work)
            eng_load = {"scalar": 0.0, "dve": 0.0}
            COST_S = 9.3e-3  # us per kilo-element on ScalarE (penalized: measured busier)
            COST_V = 10.0e-3  # us per kilo-element on VectorE
            PAIR_KELS = 2 * QB * 128 / 1000.0

            pending = []  # deferred epilogue: (otp, bh, qi)

            def flush_pending():
                while pending:
                    otp_, bh_, qi_ = pending.pop(0)
                    osb = osb_pool.tile([D + 1, QB], f16)
                    if eng_load["scalar"] + 0.45 <= eng_load["dve"] + 0.5:
                        eng_load["scalar"] += 0.45
                        nc.scalar.copy(osb[:], otp_[:])
                    else:
                        eng_load["dve"] += 0.5
                        nc.vector.tensor_copy(osb[:], otp_[:])
                    nc.sync.dma_start(out_d[bh_, qi_], osb[:])

            def emit_exp(ex_ap, stp_ap, kels, forced_scalar):
                if forced_scalar or (
                    eng_load["scalar"] + COST_S * kels
                    <= eng_load["dve"] + COST_V * kels
                ):
                    eng_load["scalar"] += COST_S * kels
                    nc.scalar.activation(
                        ex_ap, stp_ap, Exp, bias=ln4[:], scale=SCALE
                    )
                else:
                    eng_load["dve"] += COST_V * kels
                    nc.vector.tensor_scalar(
                        ex_ap.bitcast(i16), stp_ap, SCHR_A, SCHR_B, mult, add
                    )

            for bh in range(BHPC):
                qTt = qT_pool.tile([128, L], f16)
                kTt = kT_pool.tile([128, L], f16)
                vpt = vp_pool.tile([128, NKB, D + 1], f16)
                if bh == 0:
                    # first-needed slices on the scalar HWDGE queue - the
                    # scalar preamble clears earlier than sync's
                    nc.scalar.dma_start(kTt[:, 0:256], kT_d[bh, :, 0:256])
                    nc.scalar.dma_start(qTt[:, 0:QB], qT_d[bh, :, 0:QB])
                    nc.scalar.dma_start(kTt[:, 256:QB], kT_d[bh, :, 256:QB])
                    nc.sync.dma_start(
                        vpt[:], vp_d[bh].rearrange("p (n d) -> p n d", d=D + 1)
                    )
                    nc.sync.dma_start(tri[:], tri_d)
                    nc.sync.dma_start(qTt[:, QB:2 * QB], qT_d[bh, :, QB:2 * QB])
                    nc.sync.dma_start(kTt[:, QB:2 * QB], kT_d[bh, :, QB:2 * QB])
                    nc.sync.dma_start(qTt[:, 2 * QB:3 * QB], qT_d[bh, :, 2 * QB:3 * QB])
                    nc.sync.dma_start(kTt[:, 2 * QB:3 * QB], kT_d[bh, :, 2 * QB:3 * QB])
                    nc.sync.dma_start(qTt[:, 3 * QB:L], qT_d[bh, :, 3 * QB:L])
                    nc.sync.dma_start(kTt[:, 3 * QB:L], kT_d[bh, :, 3 * QB:L])
                else:
                    nc.sync.dma_start(kTt[:], kT_d[bh])
                    nc.sync.dma_start(qTt[:], qT_d[bh])
                    nc.sync.dma_start(
                        vpt[:], vp_d[bh].rearrange("p (n d) -> p n d", d=D + 1)
                    )

                for qi in range(NQB):
                    otp = otp_pool.tile([D + 1, QB], f32)
                    nfull = KB_PER_QB * qi  # fully-unmasked k-blocks

                    stages = [("pair", j0) for j0 in range(0, nfull, 2)]
                    stages += [("dpair", r0) for r0 in range(0, KB_PER_QB, 2)]

                    def emit_s(stage):
                        kind, a = stage
                        stp = stp_pool.tile([128, 2 * QB], f32, tag="stp")
                        ex = ex_pool.tile([128, 2 * QB], f16, tag="ex")
                        if kind == "pair":
                            for h_ in (0, 1):
                                j = a + h_
                                nc.tensor.matmul(
                                    stp[:, h_ * QB : (h_ + 1) * QB],
                                    lhsT=kTt[:, j * KB : (j + 1) * KB],
                                    rhs=qTt[:, qi * QB : (qi + 1) * QB],
                                    start=True,
                                    stop=True,
                                    skip_group_check=True,
                                )
                            emit_exp(ex[:], stp[:], PAIR_KELS, False)
                        else:
                            # two diagonal blocks r0, r0+1 packed into one
                            # tile: [0:na) for r0, [na:na+nb) for r0+1
                            off = 0
                            for r_ in (a, a + 1):
                                j = nfull + r_
                                m = KB * r_
                                n = QB - m
                                nc.tensor.matmul(
                                    stp[:, off : off + n],
                                    lhsT=kTt[:, j * KB : (j + 1) * KB],
                                    rhs=qTt[:, qi * QB + m : (qi + 1) * QB],
                                    start=True,
                                    stop=True,
                                    skip_group_check=True,
                                )
                                off += n
                            # few-term softmax rows (q<512) need exact exp
                            emit_exp(
                                ex[:, 0:off], stp[:, 0:off],
                                off * 0.128, qi == 0,
                            )
                            # zero the causal triangles on GpSimd
                            na = QB - KB * a
                            eng_load["dve"] += 0.44
                            nc.vector.tensor_mul(ex[:, 0:KB], ex[:, 0:KB], tri[:])
                            nc.vector.tensor_mul(
                                ex[:, na : na + KB], ex[:, na : na + KB], tri[:]
                            )
                        return ex

                    def emit_av(stage, ex, first, last):
                        kind, a = stage
                        if kind == "pair":
                            for h_ in (0, 1):
                                j = a + h_
                                nc.tensor.matmul(
                                    otp[:],
                                    lhsT=vpt[:, j, :],
                                    rhs=ex[:, h_ * QB : (h_ + 1) * QB],
                                    start=first and h_ == 0,
                                    stop=last and h_ == 1,
                                    skip_group_check=True,
                                )
                        else:
                            off = 0
                            for r_ in (a, a + 1):
                                j = nfull + r_
                                m = KB * r_
                                n = QB - m
                                nc.tensor.matmul(
                                    otp[:, m:QB],
                                    lhsT=vpt[:, j, :],
                                    rhs=ex[:, off : off + n],
                                    start=first and r_ == a,
                                    stop=last and r_ == a + 1,
                                    skip_group_check=True,
                                )
                                off += n

                    # software pipeline: keep PE fed with S-matmuls while the
                    # scalar/vector engines compute exp of earlier tiles
                    LAG = 2
                    nst = len(stages)
                    exs = {}
                    for t in range(nst + LAG):
                        if t < nst:
                            exs[t] = emit_s(stages[t])
                        if t == 0:
                            # previous block's epilogue: its last A@V long
                            # finished, so the copy never stalls the scalar
                            # queue head
                            flush_pending()
                        if t >= LAG:
                            s_ = t - LAG
                            emit_av(
                                stages[s_], exs.pop(s_),
                                first=(s_ == 0), last=(s_ == nst - 1),
                            )
                    pending.append((otp, bh, qi))
            flush_pending()

    nc.compile()
    return nc


def _host_consts():
    kk = np.arange(128)[:, None]
    cc = np.arange(128)[None, :]
    tri = (kk <= cc).astype(np.float16)
    return tri


def _shard_inputs(q, k, v, pe_q, pe_k):
    """Pure host-side layout packing (cast + transpose + concat)."""
    BH = B * H
    q = np.asarray(q, dtype=np.float32).reshape(BH, L, D)
    k = np.asarray(k, dtype=np.float32).reshape(BH, L, D)
    v = np.asarray(v, dtype=np.float32).reshape(BH, L, D)
    pe_q = np.asarray(pe_q, dtype=np.float32).reshape(BH, L, D)
    pe_k = np.asarray(pe_k, dtype=np.float32).reshape(BH, L, D)

    qT = np.ascontiguousarray(
        np.concatenate([q, pe_q], axis=-1).transpose(0, 2, 1)
    ).astype(np.float16)  # [BH, 128, L]
    kT = np.ascontiguousarray(
        np.concatenate([k, pe_k], axis=-1).transpose(0, 2, 1)
    ).astype(np.float16)

    v16 = v.astype(np.float16).reshape(BH, NKB, 128, D)
    vp = np.empty((BH, 128, NKB, D + 1), dtype=np.float16)
    vp[..., :D] = v16.transpose(0, 2, 1, 3)
    vp[..., D] = 1.0
    vp = vp.reshape(BH, 128, NKB * (D + 1))

    tri = _host_consts()
    in_maps = []
    for c in range(NCORES):
        s = slice(c * BHPC, (c + 1) * BHPC)
        in_maps.append({"qT": qT[s], "kT": kT[s], "vp": vp[s], "tri": tri})
    return in_maps


def _postprocess(per_core_out):
    """per_core_out: list of [BHPC, NQB, D+1, QB] f16 -> [B, H, L, D] f32."""
    o = np.concatenate(
        [np.asarray(x, dtype=np.float32) for x in per_core_out], axis=0
    )  # [BH, NQB, 65, QB]
    num = o[:, :, :D, :]  # [BH, NQB, D, QB]
    den = o[:, :, D, :]  # [BH, NQB, QB]
    res = (num / den[:, :, None, :]).transpose(0, 1, 3, 2)  # [BH, NQB, QB, D]
    return np.ascontiguousarray(res.reshape(B, H, L, D))


def kernel(q, k, v, pe_q, pe_k, mask=None, **_ignored):
    """Full-input entry point: shards across 8 NeuronCores, returns full output.

    The mask input is the (fixed) causal mask of the problem; causality is
    implemented structurally in the device kernel, so it is not shipped.
    """
    _import_concourse()
    from concourse.bass_utils import run_bass_kernel_spmd

    if "nc" not in _CACHE:
        _CACHE["nc"] = _build_nc()
    nc = _CACHE["nc"]

    in_maps = _shard_inputs(q, k, v, pe_q, pe_k)
    res = run_bass_kernel_spmd(nc, in_maps, core_ids=list(range(NCORES)))
    return _postprocess([res.results[c]["out"] for c in range(NCORES)])


# revision 20
# speedup vs baseline: 1.1662x; 1.1662x over previous
"""Dual-score causal attention on 8 Trainium2 NeuronCores.

Math (per batch*head):
    S = (q @ k.T + pe_q @ pe_k.T) * D**-0.5   == concat(q,pe_q) @ concat(k,pe_k).T * scale
    O = softmax(causal_mask(S)) @ v

Sharding: B*H = 32 pairs -> 4 per core (head/data parallel, no collectives).

Design (v6):
  - All layout work happens on the HOST: Q' = [q|pe_q] and K' = [k|pe_k] are
    concatenated, cast f16 and pre-TRANSPOSED to d-major so the device does
    zero transposes and only fully-contiguous HWDGE DMA loads.  V is packed
    [128, NKB, D+1] f16 with a ones column (row-sum trick).
  - S^T tiles [128 k x 512 q]: contraction (d'=128) and the A@V contraction
    both run over the partition axis; the ones column of V' yields softmax
    denominators from the same matmul chain.  Causally-dead k-blocks are
    skipped; diagonal blocks trim dead query columns.
  - ~28 warm-up matmuls on a memset scratch tile run during the DMA-wait
    window: they latch the PE_HAM activity monitor to 8/8 (2.4 GHz) before
    real work arrives, avoiding ~8us of half-clock matmuls.
  - exp() is split across two engines: a deficit balancer assigns each tile
    to exact-exp on ScalarE (bias ln4 matches the 2^2 Schraudolph offset) or
    Schraudolph fast-exp on VectorE (one tensor_scalar x*A+B, int16-convert
    output bit-viewed as f16: bits = round(1024*(log2e*scale*s+17)) =>
    2^t*(1+-3%)).  Only qi=0's diagonal tiles are forced exact: few-term
    softmax rows (q<512) need exact weights, elsewhere the noise averages
    out (4.8e-3 measured vs the 2e-2 gate).
  - The 128-col causal triangles are zeroed by 0/1 f16 multiplies on the
    otherwise-idle GpSimd engine.
  - Output: unnormalized O^T [65, 512] per query block (row 64 = softmax
    denominator) is copied f32->f16 (ScalarE) and DMA'd out contiguously;
    the host divides and transposes.  Epilogues are deferred one stage into
    the next query block so they never head-block the scalar engine queue.
  - The first input DMAs issue from the scalar HWDGE queue (its preamble
    clears ~3us before the sync engine's); bulk loads stay on sync.
"""

import os
import sys

import numpy as np

B, H, L, D = 2, 16, 2048, 64
NCORES = 8
BHPC = (B * H) // NCORES  # bh pairs per core = 4
QB = 512  # query block (S^T free dim)
KB = 128  # key block (S^T partition dim)
NQB = L // QB  # 4
NKB = L // KB  # 16
KB_PER_QB = QB // KB  # 4
SCALE = float(D) ** -0.5
LOG2E = 1.4426950408889634
# Schraudolph f16: bits = round(s * SCHR_A + SCHR_B); exponent offset C=2
# (all weights scaled by 4; cancels in the softmax division on host).
SCHR_A = SCALE * LOG2E * 1024.0
SCHR_B = 1024.0 * (15.0 + 2.0) - 44.2
LN4 = 1.3862943611198906  # exact-exp path matches the 2^2 offset

_CACHE = {}


def _import_concourse():
    try:
        import concourse  # noqa: F401
    except ImportError:
        for p in ("/opt/trn_rl_repo", "/root/.axon_site/_ro/trn_rl_repo"):
            if os.path.isdir(p) and p not in sys.path:
                sys.path.insert(0, p)


def _build_nc():
    """Build the single-core Bass program (same NEFF for all 8 cores)."""
    _import_concourse()
    from contextlib import ExitStack

    import concourse.tile as tile
    from concourse import bacc, mybir

    f32 = mybir.dt.float32
    f16 = mybir.dt.float16
    i16 = mybir.dt.int16
    nc = bacc.Bacc("TRN2", target_bir_lowering=False, debug=False)

    qT_d = nc.dram_tensor("qT", [BHPC, 128, L], f16, kind="ExternalInput").ap()
    kT_d = nc.dram_tensor("kT", [BHPC, 128, L], f16, kind="ExternalInput").ap()
    vp_d = nc.dram_tensor("vp", [BHPC, 128, NKB * (D + 1)], f16, kind="ExternalInput").ap()
    tri_d = nc.dram_tensor("tri", [128, 128], f16, kind="ExternalInput").ap()
    out_d = nc.dram_tensor("out", [BHPC, NQB, D + 1, QB], f16, kind="ExternalOutput").ap()

    Exp = mybir.ActivationFunctionType.Exp
    mult = mybir.AluOpType.mult
    add = mybir.AluOpType.add

    with tile.TileContext(nc) as tc:
        with ExitStack() as ctx:
            ep = ctx.enter_context

            const_pool = ep(tc.tile_pool(name="const", bufs=1))
            qT_pool = ep(tc.tile_pool(name="qT", bufs=2))
            kT_pool = ep(tc.tile_pool(name="kT", bufs=2))
            vp_pool = ep(tc.tile_pool(name="vp", bufs=2))
            ex_pool = ep(tc.tile_pool(name="ex", bufs=7))
            osb_pool = ep(tc.tile_pool(name="osb", bufs=3))
            stp_pool = ep(tc.tile_pool(name="stp", bufs=3, space="PSUM"))
            otp_pool = ep(tc.tile_pool(name="otp", bufs=2, space="PSUM"))

            tri = const_pool.tile([128, 128], f16)
            ln4 = const_pool.tile([128, 1], f32)
            nc.vector.memset(ln4[:], LN4)

            # ---- PE warm-up: latch HAM to 8/8 while DMAs are in flight ----
            scratch = const_pool.tile([128, 128], f16)
            nc.gpsimd.memset(scratch[:], 0.0)
            wup = stp_pool.tile([128, 2 * QB], f32, tag="stp")
            for w in range(64):
                nc.tensor.matmul(
                    wup[:, 0:128],
                    lhsT=scratch[:],
                    rhs=scratch[:],
                    start=True,
                    stop=True,
                    skip_group_check=True,
                )

            # deficit balancer for exp engine assignment (us of est. work)
            eng_load = {"scalar": 0.0, "dve": 0.0}
            COST_S = 8.0e-3  # us per kilo-element on ScalarE
            COST_V = 10.0e-3  # us per kilo-element on VectorE
            PAIR_KELS = 2 * QB * 128 / 1000.0

            pending = []  # deferred epilogue: (otp, bh, qi)

            def flush_pending():
                while pending:
                    otp_, bh_, qi_ = pending.pop(0)
                    osb = osb_pool.tile([D + 1, QB], f16)
                    if eng_load["scalar"] + 0.45 <= eng_load["dve"] + 0.5:
                        eng_load["scalar"] += 0.45
                        nc.scalar.copy(osb[:], otp_[:])
                    else:
                        eng_load["dve"] += 0.5
                        nc.vector.tensor_copy(osb[:], otp_[:])
                    nc.sync.dma_start(out_d[bh_, qi_], osb[:])

            def emit_exp(ex_ap, stp_ap, kels, forced_scalar):
                if forced_scalar or (
                    eng_load["scalar"] + COST_S * kels
                    <= eng_load["dve"] + COST_V * kels
                ):
                    eng_load["scalar"] += COST_S * kels
                    nc.scalar.activation(
                        ex_ap, stp_ap, Exp, bias=ln4[:], scale=SCALE
                    )
                else:
                    eng_load["dve"] += COST_V * kels
                    nc.vector.tensor_scalar(
                        ex_ap.bitcast(i16), stp_ap, SCHR_A, SCHR_B, mult, add
                    )

            for bh in range(BHPC):
                qTt = qT_pool.tile([128, L], f16)
                kTt = kT_pool.tile([128, L], f16)
                vpt = vp_pool.tile([128, NKB, D + 1], f16)
                if bh == 0:
                    # first-needed slices on the scalar HWDGE queue - the
                    # scalar preamble clears earlier than sync's
                    nc.scalar.dma_start(kTt[:, 0:256], kT_d[bh, :, 0:256])
                    nc.scalar.dma_start(qTt[:, 0:QB], qT_d[bh, :, 0:QB])
                    nc.scalar.dma_start(kTt[:, 256:QB], kT_d[bh, :, 256:QB])
                    nc.sync.dma_start(
                        vpt[:], vp_d[bh].rearrange("p (n d) -> p n d", d=D + 1)
                    )
                    nc.sync.dma_start(tri[:], tri_d)
                    nc.sync.dma_start(qTt[:, QB:2 * QB], qT_d[bh, :, QB:2 * QB])
                    nc.sync.dma_start(kTt[:, QB:2 * QB], kT_d[bh, :, QB:2 * QB])
                    nc.sync.dma_start(qTt[:, 2 * QB:3 * QB], qT_d[bh, :, 2 * QB:3 * QB])
                    nc.sync.dma_start(kTt[:, 2 * QB:3 * QB], kT_d[bh, :, 2 * QB:3 * QB])
                    nc.sync.dma_start(qTt[:, 3 * QB:L], qT_d[bh, :, 3 * QB:L])
                    nc.sync.dma_start(kTt[:, 3 * QB:L], kT_d[bh, :, 3 * QB:L])
                else:
                    nc.sync.dma_start(kTt[:], kT_d[bh])
                    nc.sync.dma_start(qTt[:], qT_d[bh])
                    nc.sync.dma_start(
                        vpt[:], vp_d[bh].rearrange("p (n d) -> p n d", d=D + 1)
                    )

                for qi in range(NQB):
                    otp = otp_pool.tile([D + 1, QB], f32)
                    nfull = KB_PER_QB * qi  # fully-unmasked k-blocks

                    stages = [("pair", j0) for j0 in range(0, nfull, 2)]
                    stages += [("dpair", r0) for r0 in range(0, KB_PER_QB, 2)]

                    def emit_s(stage):
                        kind, a = stage
                        stp = stp_pool.tile([128, 2 * QB], f32, tag="stp")
                        ex = ex_pool.tile([128, 2 * QB], f16, tag="ex")
                        if kind == "pair":
                            for h_ in (0, 1):
                                j = a + h_
                                nc.tensor.matmul(
                                    stp[:, h_ * QB : (h_ + 1) * QB],
                                    lhsT=kTt[:, j * KB : (j + 1) * KB],
                                    rhs=qTt[:, qi * QB : (qi + 1) * QB],
                                    start=True,
                                    stop=True,
                                    skip_group_check=True,
                                )
                            emit_exp(ex[:], stp[:], PAIR_KELS, False)
                        else:
                            # two diagonal blocks r0, r0+1 packed into one
                            # tile: [0:na) for r0, [na:na+nb) for r0+1
                            off = 0
                            for r_ in (a, a + 1):
                                j = nfull + r_
                                m = KB * r_
                                n = QB - m
                                nc.tensor.matmul(
                                    stp[:, off : off + n],
                                    lhsT=kTt[:, j * KB : (j + 1) * KB],
                                    rhs=qTt[:, qi * QB + m : (qi + 1) * QB],
                                    start=True,
                                    stop=True,
                                    skip_group_check=True,
                                )
                                off += n
                            # few-term softmax rows (q<512) need exact exp
                            emit_exp(
                                ex[:, 0:off], stp[:, 0:off],
                                off * 0.128, qi == 0,
                            )
                            # zero the causal triangles on GpSimd
                            na = QB - KB * a
                            eng_load["dve"] += 0.44
                            nc.vector.tensor_mul(ex[:, 0:KB], ex[:, 0:KB], tri[:])
                            nc.vector.tensor_mul(
                                ex[:, na : na + KB], ex[:, na : na + KB], tri[:]
                            )
                        return ex

                    def emit_av(stage, ex, first, last):
                        kind, a = stage
                        if kind == "pair":
                            for h_ in (0, 1):
                                j = a + h_
                                nc.tensor.matmul(
                                    otp[:],
                                    lhsT=vpt[:, j, :],
                                    rhs=ex[:, h_ * QB : (h_ + 1) * QB],
                                    start=first and h_ == 0,
                                    stop=last and h_ == 1,
                                    skip_group_check=True,
                                )
                        else:
                            off = 0
                            for r_ in (a, a + 1):
                                j = nfull + r_
                                m = KB * r_
                                n = QB - m
                                nc.tensor.matmul(
                                    otp[:, m:QB],
                                    lhsT=vpt[:, j, :],
                                    rhs=ex[:, off : off + n],
                                    start=first and r_ == a,
                                    stop=last and r_ == a + 1,
                                    skip_group_check=True,
                                )
                                off += n

                    # software pipeline: keep PE fed with S-matmuls while the
                    # scalar/vector engines compute exp of earlier tiles
                    LAG = 2
                    nst = len(stages)
                    exs = {}
                    for t in range(nst + LAG):
                        if t < nst:
                            exs[t] = emit_s(stages[t])
                        if t == 0:
                            # previous block's epilogue: its last A@V long
                            # finished, so the copy never stalls the scalar
                            # queue head
                            flush_pending()
                        if t >= LAG:
                            s_ = t - LAG
                            emit_av(
                                stages[s_], exs.pop(s_),
                                first=(s_ == 0), last=(s_ == nst - 1),
                            )
                    pending.append((otp, bh, qi))
            flush_pending()

    nc.compile()
    return nc


def _host_consts():
    kk = np.arange(128)[:, None]
    cc = np.arange(128)[None, :]
    tri = (kk <= cc).astype(np.float16)
    return tri


def _shard_inputs(q, k, v, pe_q, pe_k):
    """Pure host-side layout packing (cast + transpose + concat)."""
    BH = B * H
    q = np.asarray(q, dtype=np.float32).reshape(BH, L, D)
    k = np.asarray(k, dtype=np.float32).reshape(BH, L, D)
    v = np.asarray(v, dtype=np.float32).reshape(BH, L, D)
    pe_q = np.asarray(pe_q, dtype=np.float32).reshape(BH, L, D)
    pe_k = np.asarray(pe_k, dtype=np.float32).reshape(BH, L, D)

    qT = np.ascontiguousarray(
        np.concatenate([q, pe_q], axis=-1).transpose(0, 2, 1)
    ).astype(np.float16)  # [BH, 128, L]
    kT = np.ascontiguousarray(
        np.concatenate([k, pe_k], axis=-1).transpose(0, 2, 1)
    ).astype(np.float16)

    v16 = v.astype(np.float16).reshape(BH, NKB, 128, D)
    vp = np.empty((BH, 128, NKB, D + 1), dtype=np.float16)
    vp[..., :D] = v16.transpose(0, 2, 1, 3)
    vp[..., D] = 1.0
    vp = vp.reshape(BH, 128, NKB * (D + 1))

    tri = _host_consts()
    in_maps = []
    for c in range(NCORES):
        s = slice(c * BHPC, (c + 1) * BHPC)
        in_maps.append({"qT": qT[s], "kT": kT[s], "vp": vp[s], "tri": tri})
    return in_maps


def _postprocess(per_core_out):
    """per_core_out: list of [BHPC, NQB, D+1, QB] f16 -> [B, H, L, D] f32."""
    o = np.concatenate(
        [np.asarray(x, dtype=np.float32) for x in per_core_out], axis=0
    )  # [BH, NQB, 65, QB]
    num = o[:, :, :D, :]  # [BH, NQB, D, QB]
    den = o[:, :, D, :]  # [BH, NQB, QB]
    res = (num / den[:, :, None, :]).transpose(0, 1, 3, 2)  # [BH, NQB, QB, D]
    return np.ascontiguousarray(res.reshape(B, H, L, D))


def kernel(q, k, v, pe_q, pe_k, mask=None, **_ignored):
    """Full-input entry point: shards across 8 NeuronCores, returns full output.

    The mask input is the (fixed) causal mask of the problem; causality is
    implemented structurally in the device kernel, so it is not shipped.
    """
    _import_concourse()
    from concourse.bass_utils import run_bass_kernel_spmd

    if "nc" not in _CACHE:
        _CACHE["nc"] = _build_nc()
    nc = _CACHE["nc"]

    in_maps = _shard_inputs(q, k, v, pe_q, pe_k)
    res = run_bass_kernel_spmd(nc, in_maps, core_ids=list(range(NCORES)))
    return _postprocess([res.results[c]["out"] for c in range(NCORES)])


# revision 21
# speedup vs baseline: 1.1682x; 1.0017x over previous
"""Dual-score causal attention on 8 Trainium2 NeuronCores.

Math (per batch*head):
    S = (q @ k.T + pe_q @ pe_k.T) * D**-0.5   == concat(q,pe_q) @ concat(k,pe_k).T * scale
    O = softmax(causal_mask(S)) @ v

Sharding: B*H = 32 pairs -> 4 per core (head/data parallel, no collectives).

Design (v6):
  - All layout work happens on the HOST: Q' = [q|pe_q] and K' = [k|pe_k] are
    concatenated, cast f16 and pre-TRANSPOSED to d-major so the device does
    zero transposes and only fully-contiguous HWDGE DMA loads.  V is packed
    [128, NKB, D+1] f16 with a ones column (row-sum trick).
  - S^T tiles [128 k x 512 q]: contraction (d'=128) and the A@V contraction
    both run over the partition axis; the ones column of V' yields softmax
    denominators from the same matmul chain.  Causally-dead k-blocks are
    skipped; diagonal blocks trim dead query columns.
  - ~28 warm-up matmuls on a memset scratch tile run during the DMA-wait
    window: they latch the PE_HAM activity monitor to 8/8 (2.4 GHz) before
    real work arrives, avoiding ~8us of half-clock matmuls.
  - exp() is split across two engines: a deficit balancer assigns each tile
    to exact-exp on ScalarE (bias ln4 matches the 2^2 Schraudolph offset) or
    Schraudolph fast-exp on VectorE (one tensor_scalar x*A+B, int16-convert
    output bit-viewed as f16: bits = round(1024*(log2e*scale*s+17)) =>
    2^t*(1+-3%)).  Only qi=0's diagonal tiles are forced exact: few-term
    softmax rows (q<512) need exact weights, elsewhere the noise averages
    out (4.8e-3 measured vs the 2e-2 gate).
  - The 128-col causal triangles are zeroed by 0/1 f16 multiplies on the
    otherwise-idle GpSimd engine.
  - Output: unnormalized O^T [65, 512] per query block (row 64 = softmax
    denominator) is copied f32->f16 (ScalarE) and DMA'd out contiguously;
    the host divides and transposes.  Epilogues are deferred one stage into
    the next query block so they never head-block the scalar engine queue.
  - The first input DMAs issue from the scalar HWDGE queue (its preamble
    clears ~3us before the sync engine's); bulk loads stay on sync.
"""

import os
import sys

import numpy as np

B, H, L, D = 2, 16, 2048, 64
NCORES = 8
BHPC = (B * H) // NCORES  # bh pairs per core = 4
QB = 512  # query block (S^T free dim)
KB = 128  # key block (S^T partition dim)
NQB = L // QB  # 4
NKB = L // KB  # 16
KB_PER_QB = QB // KB  # 4
SCALE = float(D) ** -0.5
LOG2E = 1.4426950408889634
# Schraudolph f16: bits = round(s * SCHR_A + SCHR_B); exponent offset C=2
# (all weights scaled by 4; cancels in the softmax division on host).
SCHR_A = SCALE * LOG2E * 1024.0
SCHR_B = 1024.0 * (15.0 + 2.0) - 44.2
LN4 = 1.3862943611198906  # exact-exp path matches the 2^2 offset

_CACHE = {}


def _import_concourse():
    try:
        import concourse  # noqa: F401
    except ImportError:
        for p in ("/opt/trn_rl_repo", "/root/.axon_site/_ro/trn_rl_repo"):
            if os.path.isdir(p) and p not in sys.path:
                sys.path.insert(0, p)


def _build_nc():
    """Build the single-core Bass program (same NEFF for all 8 cores)."""
    _import_concourse()
    from contextlib import ExitStack

    import concourse.tile as tile
    from concourse import bacc, mybir

    f32 = mybir.dt.float32
    f16 = mybir.dt.float16
    i16 = mybir.dt.int16
    nc = bacc.Bacc("TRN2", target_bir_lowering=False, debug=False)

    qT_d = nc.dram_tensor("qT", [BHPC, 128, L], f16, kind="ExternalInput").ap()
    kT_d = nc.dram_tensor("kT", [BHPC, 128, L], f16, kind="ExternalInput").ap()
    vp_d = nc.dram_tensor("vp", [BHPC, 128, NKB * (D + 1)], f16, kind="ExternalInput").ap()
    tri_d = nc.dram_tensor("tri", [128, 128], f16, kind="ExternalInput").ap()
    out_d = nc.dram_tensor("out", [BHPC, NQB, D + 1, QB], f16, kind="ExternalOutput").ap()

    Exp = mybir.ActivationFunctionType.Exp
    mult = mybir.AluOpType.mult
    add = mybir.AluOpType.add

    with tile.TileContext(nc) as tc:
        with ExitStack() as ctx:
            ep = ctx.enter_context

            const_pool = ep(tc.tile_pool(name="const", bufs=1))
            qT_pool = ep(tc.tile_pool(name="qT", bufs=2))
            kT_pool = ep(tc.tile_pool(name="kT", bufs=2))
            vp_pool = ep(tc.tile_pool(name="vp", bufs=2))
            ex_pool = ep(tc.tile_pool(name="ex", bufs=7))
            osb_pool = ep(tc.tile_pool(name="osb", bufs=3))
            stp_pool = ep(tc.tile_pool(name="stp", bufs=3, space="PSUM"))
            otp_pool = ep(tc.tile_pool(name="otp", bufs=2, space="PSUM"))

            tri = const_pool.tile([128, 128], f16)
            ln4 = const_pool.tile([128, 1], f32)
            nc.vector.memset(ln4[:], LN4)

            # ---- PE warm-up: latch HAM to 8/8 while DMAs are in flight ----
            scratch = const_pool.tile([128, 128], f16)
            nc.gpsimd.memset(scratch[:], 0.0)
            wup = stp_pool.tile([128, 2 * QB], f32, tag="stp")
            for w in range(56):
                nc.tensor.matmul(
                    wup[:, 0:128],
                    lhsT=scratch[:],
                    rhs=scratch[:],
                    start=True,
                    stop=True,
                    skip_group_check=True,
                )

            # deficit balancer for exp engine assignment (us of est. work)
            eng_load = {"scalar": 0.0, "dve": 0.0}
            COST_S = 8.0e-3  # us per kilo-element on ScalarE
            COST_V = 10.0e-3  # us per kilo-element on VectorE
            PAIR_KELS = 2 * QB * 128 / 1000.0

            pending = []  # deferred epilogue: (otp, bh, qi)

            def flush_pending():
                while pending:
                    otp_, bh_, qi_ = pending.pop(0)
                    osb = osb_pool.tile([D + 1, QB], f16)
                    if eng_load["scalar"] + 0.45 <= eng_load["dve"] + 0.5:
                        eng_load["scalar"] += 0.45
                        nc.scalar.copy(osb[:], otp_[:])
                    else:
                        eng_load["dve"] += 0.5
                        nc.vector.tensor_copy(osb[:], otp_[:])
                    nc.sync.dma_start(out_d[bh_, qi_], osb[:])

            def emit_exp(ex_ap, stp_ap, kels, forced_scalar):
                if forced_scalar or (
                    eng_load["scalar"] + COST_S * kels
                    <= eng_load["dve"] + COST_V * kels
                ):
                    eng_load["scalar"] += COST_S * kels
                    nc.scalar.activation(
                        ex_ap, stp_ap, Exp, bias=ln4[:], scale=SCALE
                    )
                else:
                    eng_load["dve"] += COST_V * kels
                    nc.vector.tensor_scalar(
                        ex_ap.bitcast(i16), stp_ap, SCHR_A, SCHR_B, mult, add
                    )

            for bh in range(BHPC):
                qTt = qT_pool.tile([128, L], f16)
                kTt = kT_pool.tile([128, L], f16)
                vpt = vp_pool.tile([128, NKB, D + 1], f16)
                if bh == 0:
                    # first-needed slices on the scalar HWDGE queue - the
                    # scalar preamble clears earlier than sync's
                    nc.scalar.dma_start(kTt[:, 0:256], kT_d[bh, :, 0:256])
                    nc.scalar.dma_start(qTt[:, 0:QB], qT_d[bh, :, 0:QB])
                    nc.scalar.dma_start(kTt[:, 256:QB], kT_d[bh, :, 256:QB])
                    nc.sync.dma_start(
                        vpt[:], vp_d[bh].rearrange("p (n d) -> p n d", d=D + 1)
                    )
                    nc.sync.dma_start(tri[:], tri_d)
                    nc.sync.dma_start(qTt[:, QB:2 * QB], qT_d[bh, :, QB:2 * QB])
                    nc.sync.dma_start(kTt[:, QB:2 * QB], kT_d[bh, :, QB:2 * QB])
                    nc.sync.dma_start(qTt[:, 2 * QB:3 * QB], qT_d[bh, :, 2 * QB:3 * QB])
                    nc.sync.dma_start(kTt[:, 2 * QB:3 * QB], kT_d[bh, :, 2 * QB:3 * QB])
                    nc.sync.dma_start(qTt[:, 3 * QB:L], qT_d[bh, :, 3 * QB:L])
                    nc.sync.dma_start(kTt[:, 3 * QB:L], kT_d[bh, :, 3 * QB:L])
                else:
                    nc.sync.dma_start(kTt[:], kT_d[bh])
                    nc.sync.dma_start(qTt[:], qT_d[bh])
                    nc.sync.dma_start(
                        vpt[:], vp_d[bh].rearrange("p (n d) -> p n d", d=D + 1)
                    )

                for qi in range(NQB):
                    otp = otp_pool.tile([D + 1, QB], f32)
                    nfull = KB_PER_QB * qi  # fully-unmasked k-blocks

                    stages = [("pair", j0) for j0 in range(0, nfull, 2)]
                    stages += [("dpair", r0) for r0 in range(0, KB_PER_QB, 2)]

                    def emit_s(stage):
                        kind, a = stage
                        stp = stp_pool.tile([128, 2 * QB], f32, tag="stp")
                        ex = ex_pool.tile([128, 2 * QB], f16, tag="ex")
                        if kind == "pair":
                            for h_ in (0, 1):
                                j = a + h_
                                nc.tensor.matmul(
                                    stp[:, h_ * QB : (h_ + 1) * QB],
                                    lhsT=kTt[:, j * KB : (j + 1) * KB],
                                    rhs=qTt[:, qi * QB : (qi + 1) * QB],
                                    start=True,
                                    stop=True,
                                    skip_group_check=True,
                                )
                            emit_exp(ex[:], stp[:], PAIR_KELS, False)
                        else:
                            # two diagonal blocks r0, r0+1 packed into one
                            # tile: [0:na) for r0, [na:na+nb) for r0+1
                            off = 0
                            for r_ in (a, a + 1):
                                j = nfull + r_
                                m = KB * r_
                                n = QB - m
                                nc.tensor.matmul(
                                    stp[:, off : off + n],
                                    lhsT=kTt[:, j * KB : (j + 1) * KB],
                                    rhs=qTt[:, qi * QB + m : (qi + 1) * QB],
                                    start=True,
                                    stop=True,
                                    skip_group_check=True,
                                )
                                off += n
                            # few-term softmax rows (q<512) need exact exp
                            emit_exp(
                                ex[:, 0:off], stp[:, 0:off],
                                off * 0.128, qi == 0,
                            )
                            # zero the causal triangles on GpSimd
                            na = QB - KB * a
                            eng_load["dve"] += 0.44
                            nc.vector.tensor_mul(ex[:, 0:KB], ex[:, 0:KB], tri[:])
                            nc.vector.tensor_mul(
                                ex[:, na : na + KB], ex[:, na : na + KB], tri[:]
                            )
                        return ex

                    def emit_av(stage, ex, first, last):
                        kind, a = stage
                        if kind == "pair":
                            for h_ in (0, 1):
                                j = a + h_
                                nc.tensor.matmul(
                                    otp[:],
                                    lhsT=vpt[:, j, :],
                                    rhs=ex[:, h_ * QB : (h_ + 1) * QB],
                                    start=first and h_ == 0,
                                    stop=last and h_ == 1,
                                    skip_group_check=True,
                                )
                        else:
                            off = 0
                            for r_ in (a, a + 1):
                                j = nfull + r_
                                m = KB * r_
                                n = QB - m
                                nc.tensor.matmul(
                                    otp[:, m:QB],
                                    lhsT=vpt[:, j, :],
                                    rhs=ex[:, off : off + n],
                                    start=first and r_ == a,
                                    stop=last and r_ == a + 1,
                                    skip_group_check=True,
                                )
                                off += n

                    # software pipeline: keep PE fed with S-matmuls while the
                    # scalar/vector engines compute exp of earlier tiles
                    LAG = 2
                    nst = len(stages)
                    exs = {}
                    for t in range(nst + LAG):
                        if t < nst:
                            exs[t] = emit_s(stages[t])
                        if t == 0:
                            # previous block's epilogue: its last A@V long
                            # finished, so the copy never stalls the scalar
                            # queue head
                            flush_pending()
                        if t >= LAG:
                            s_ = t - LAG
                            emit_av(
                                stages[s_], exs.pop(s_),
                                first=(s_ == 0), last=(s_ == nst - 1),
                            )
                    pending.append((otp, bh, qi))
            flush_pending()

    nc.compile()
    return nc


def _host_consts():
    kk = np.arange(128)[:, None]
    cc = np.arange(128)[None, :]
    tri = (kk <= cc).astype(np.float16)
    return tri


def _shard_inputs(q, k, v, pe_q, pe_k):
    """Pure host-side layout packing (cast + transpose + concat)."""
    BH = B * H
    q = np.asarray(q, dtype=np.float32).reshape(BH, L, D)
    k = np.asarray(k, dtype=np.float32).reshape(BH, L, D)
    v = np.asarray(v, dtype=np.float32).reshape(BH, L, D)
    pe_q = np.asarray(pe_q, dtype=np.float32).reshape(BH, L, D)
    pe_k = np.asarray(pe_k, dtype=np.float32).reshape(BH, L, D)

    qT = np.ascontiguousarray(
        np.concatenate([q, pe_q], axis=-1).transpose(0, 2, 1)
    ).astype(np.float16)  # [BH, 128, L]
    kT = np.ascontiguousarray(
        np.concatenate([k, pe_k], axis=-1).transpose(0, 2, 1)
    ).astype(np.float16)

    v16 = v.astype(np.float16).reshape(BH, NKB, 128, D)
    vp = np.empty((BH, 128, NKB, D + 1), dtype=np.float16)
    vp[..., :D] = v16.transpose(0, 2, 1, 3)
    vp[..., D] = 1.0
    vp = vp.reshape(BH, 128, NKB * (D + 1))

    tri = _host_consts()
    in_maps = []
    for c in range(NCORES):
        s = slice(c * BHPC, (c + 1) * BHPC)
        in_maps.append({"qT": qT[s], "kT": kT[s], "vp": vp[s], "tri": tri})
    return in_maps


def _postprocess(per_core_out):
    """per_core_out: list of [BHPC, NQB, D+1, QB] f16 -> [B, H, L, D] f32."""
    o = np.concatenate(
        [np.asarray(x, dtype=np.float32) for x in per_core_out], axis=0
    )  # [BH, NQB, 65, QB]
    num = o[:, :, :D, :]  # [BH, NQB, D, QB]
    den = o[:, :, D, :]  # [BH, NQB, QB]
    res = (num / den[:, :, None, :]).transpose(0, 1, 3, 2)  # [BH, NQB, QB, D]
    return np.ascontiguousarray(res.reshape(B, H, L, D))


def kernel(q, k, v, pe_q, pe_k, mask=None, **_ignored):
    """Full-input entry point: shards across 8 NeuronCores, returns full output.

    The mask input is the (fixed) causal mask of the problem; causality is
    implemented structurally in the device kernel, so it is not shipped.
    """
    _import_concourse()
    from concourse.bass_utils import run_bass_kernel_spmd

    if "nc" not in _CACHE:
        _CACHE["nc"] = _build_nc()
    nc = _CACHE["nc"]

    in_maps = _shard_inputs(q, k, v, pe_q, pe_k)
    res = run_bass_kernel_spmd(nc, in_maps, core_ids=list(range(NCORES)))
    return _postprocess([res.results[c]["out"] for c in range(NCORES)])
